# revision 1
# baseline (speedup 1.0000x reference)
"""MoE MLP block (RMSNorm + top-2 router + 8-expert GLU MLP) on 8 TRN2 cores.

Strategy: expert parallelism, one expert per core.
  - Each core computes the router for its 1/8 slice of tokens (RMSNorm stats +
    logits + top-2 + normalized weights), then AllGathers the tiny routing
    table so every core knows every token's (e1, e2, w1, w2, rms_scale).
  - Each core builds dispatch metadata for its own expert fully on-device
    (prefix-sum via DVE scan + a strict-triangular matmul; slot->token map via
    a one-hot matmul), indirect-DMA-gathers its tokens' rows of x, applies
    RMSNorm, transposes to put H on partitions, and runs the expert GLU MLP
    as float32r matmuls (full PE rate at N>=256, ~1e-4 relative error).
  - Weighted outputs are indirect-DMA-scattered into a zeroed [T, H]
    contribution buffer; a ReduceScatter(add) across the 8 cores yields each
    core's 1/8 shard of the final output, which the host concatenates.
"""
import sys
sys.path.insert(0, '/opt/trn_rl_repo')
import numpy as np

# ---- problem constants (hardcoded per contract) ----
B, S, H, I, E = 2, 1024, 2048, 4096, 8
T = B * S                    # 2048 tokens
EPS = 1e-6
NCORES = 8
KH = H // 128                # 16 h-tiles
KI = I // 128                # 32 i-tiles
CAP = 576                    # max tokens per expert (seed-0 max count is 545)
NST = (CAP + 127) // 128     # 5 slot tiles
ST_W = [min(128, CAP - st * 128) for st in range(NST)]   # 128,128,128,128,64
SCH = 2                      # gate/up slot chunks
CHW = CAP // SCH             # 288 per chunk
NH = 4                       # down-proj h chunks of 512
TSL = T // NCORES            # 256 tokens per core's router slice

_CACHE = {}


def _build():
    from concourse import bass, mybir
    import concourse.bacc as bacc
    import concourse.tile as tile
    from concourse.masks import make_identity

    dt = mybir.dt
    f32, f32r, i32, u32 = dt.float32, dt.float32r, dt.int32, dt.uint32
    Alu = mybir.AluOpType
    Act = mybir.ActivationFunctionType

    nc = bacc.Bacc("TRN2", target_bir_lowering=False, debug=False,
                   num_devices=NCORES)

    x_d = nc.dram_tensor("x", [T, H], f32, kind="ExternalInput").ap()
    xs_d = nc.dram_tensor("x_slice", [TSL, H], f32, kind="ExternalInput").ap()
    nw_d = nc.dram_tensor("norm_w", [H], f32, kind="ExternalInput").ap()
    rw_d = nc.dram_tensor("router_w", [H, E], f32, kind="ExternalInput").ap()
    wg_d = nc.dram_tensor("wg", [H, I], f32, kind="ExternalInput").ap()
    wu_d = nc.dram_tensor("wu", [H, I], f32, kind="ExternalInput").ap()
    wd_d = nc.dram_tensor("wd", [I, H], f32, kind="ExternalInput").ap()
    eid_d = nc.dram_tensor("eid", [128, 1], f32, kind="ExternalInput").ap()
    out_d = nc.dram_tensor("out_shard", [TSL, H], f32, kind="ExternalOutput").ap()

    with tile.TileContext(nc) as tc:
        with tc.tile_pool(name="cst", bufs=1) as cst, \
             tc.tile_pool(name="sb", bufs=2) as sb, \
             tc.tile_pool(name="big", bufs=1) as big, \
             tc.tile_pool(name="wp", bufs=2) as wp, \
             tc.tile_pool(name="psA", bufs=6, space="PSUM") as psA, \
             tc.tile_pool(name="psB", bufs=2, space="PSUM") as psB, \
             tc.tile_pool(name="dram", bufs=1, space="DRAM") as dram:

            # ============ DRAM scratch ============
            contrib = dram.tile([T, H], f32)
            rt_slice = dram.tile([TSL, 5], f32)
            rt_full = dram.tile([T, 5], f32)
            rs_out = dram.tile([TSL, H], f32)

            # ============ constants ============
            ident = cst.tile([128, 128], f32)
            make_identity(nc, ident[:])
            tri = cst.tile([128, 128], f32)        # tri[p',p]=1 iff p'<p
            nc.gpsimd.memset(tri[:], 1.0)
            nc.gpsimd.affine_select(out=tri[:], in_=tri[:], compare_op=Alu.is_gt,
                                    fill=0.0, base=0, pattern=[[1, 128]],
                                    channel_multiplier=-1)
            eid_t = cst.tile([128, 1], f32)
            nc.sync.dma_start(eid_t[:], eid_d)
            # nwb doubles as the zero tile for contrib zero-fill, then holds
            # norm_w broadcast to all 128 partitions.
            nwb = cst.tile([128, H], f32)
            nc.vector.memset(nwb[:], 0.0)
            for c in range(T // 128):
                nc.sync.dma_start(contrib[c * 128:(c + 1) * 128, :], nwb[:])
            nc.sync.dma_start(nwb[:], nw_d.unsqueeze(0).to_broadcast([128, H]))
            iob = cst.tile([128, CAP], f32)        # each row = 0..CAP-1
            nc.gpsimd.iota(iob[:].bitcast(i32), pattern=[[1, CAP]], base=0,
                           channel_multiplier=0)
            nc.vector.tensor_copy(iob[:], iob[:].bitcast(i32))
            tval = cst.tile([128, KH], f32)        # token id at (p, c): c*128+p
            nc.gpsimd.iota(tval[:].bitcast(i32), pattern=[[128, KH]], base=0,
                           channel_multiplier=1)
            nc.vector.tensor_copy(tval[:], tval[:].bitcast(i32))
            # router weight folded with norm_w
            rw_t = sb.tile([128, KH, E], f32, tag="rw_t")
            nc.sync.dma_start(rw_t[:], rw_d.rearrange("(k p) e -> p k e", p=128))
            nw_t = sb.tile([128, KH], f32, tag="nw_t")
            nc.sync.dma_start(nw_t[:], nw_d.rearrange("(k p) -> p k", p=128))
            wp_t = cst.tile([128, KH, E], f32)
            for k in range(KH):
                nc.vector.tensor_scalar(out=wp_t[:, k, :], in0=rw_t[:, k, :],
                                        scalar1=nw_t[:, k:k + 1], scalar2=None,
                                        op0=Alu.mult)

            # ============ Phase B: router on own slice ============
            # rt columns: 0=e1 1=e2 2=w1 3=w2 4=r
            rt_s = sb.tile([128, 2, 5], f32, tag="rt_s")
            for j in range(TSL // 128):
                xsj = sb.tile([128, H], f32, tag="scr8k", bufs=3, name="xsj")
                nc.sync.dma_start(xsj[:], xs_d[j * 128:(j + 1) * 128, :])
                sq_scr = sb.tile([128, H], f32, tag="scr8k", bufs=3, name="sq_scr")
                ssq = sb.tile([128, 1], f32, tag="ssq")
                nc.scalar.activation(sq_scr[:], xsj[:], Act.Square, accum_out=ssq[:])
                var = sb.tile([128, 1], f32, tag="var")
                nc.vector.tensor_scalar(out=var[:], in0=ssq[:], scalar1=1.0 / H,
                                        scalar2=float(EPS), op0=Alu.mult, op1=Alu.add)
                sd = sb.tile([128, 1], f32, tag="sd")
                nc.scalar.sqrt(sd[:], var[:])
                r_col = sb.tile([128, 1], f32, tag="r_col")
                nc.vector.reciprocal(r_col[:], sd[:])
                # logits = x_slice @ (norm_w * router_w), via per-k transposes
                lg_ps = psB.tile([128, E], f32, tag="psmall", name="lg_ps")
                for k in range(KH):
                    xtr_ps = psA.tile([128, 128], f32, tag="pbig", name="xtr_ps")
                    nc.tensor.transpose(out=xtr_ps[:],
                                        in_=xsj[:, k * 128:(k + 1) * 128],
                                        identity=ident[:])
                    xT_k = sb.tile([128, 128], f32, tag="xT_k")
                    nc.vector.tensor_copy(xT_k[:], xtr_ps[:])
                    nc.tensor.matmul(lg_ps[:], xT_k[:], wp_t[:, k, :],
                                     start=(k == 0), stop=(k == KH - 1))
                # scaled logits s = r * logits (same top-2 as softmax affinities)
                s_t = sb.tile([128, E], f32, tag="s_t")
                nc.scalar.activation(s_t[:], lg_ps[:], Act.Copy, scale=r_col[:])
                mx = sb.tile([128, 8], f32, tag="mx")
                mi = sb.tile([128, 8], u32, tag="mi")
                nc.vector.max_with_indices(mx[:], mi[:], s_t[:])
                # w1 = 1/(1+exp(s2-s1)), w2 = 1-w1
                dlt = sb.tile([128, 1], f32, tag="dlt")
                nc.vector.tensor_sub(dlt[:], mx[:, 1:2], mx[:, 0:1])
                ew = sb.tile([128, 1], f32, tag="ew")
                nc.scalar.activation(ew[:], dlt[:], Act.Exp)
                den = sb.tile([128, 1], f32, tag="den")
                nc.vector.tensor_scalar_add(den[:], ew[:], 1.0)
                w1 = sb.tile([128, 1], f32, tag="w1")
                nc.vector.reciprocal(w1[:], den[:])
                nc.vector.tensor_copy(rt_s[:, j, 2:3], w1[:])
                nc.vector.tensor_mul(rt_s[:, j, 3:4], ew[:], w1[:])
                nc.vector.tensor_copy(rt_s[:, j, 0:2], mi[:, 0:2])
                nc.vector.tensor_copy(rt_s[:, j, 4:5], r_col[:])
            nc.sync.dma_start(rt_slice[:].rearrange("(j p) f -> p j f", p=128),
                              rt_s[:])
            nc.gpsimd.collective_compute("AllGather", Alu.bypass,
                                         replica_groups=[list(range(NCORES))],
                                         ins=[rt_slice[:]], outs=[rt_full[:]])

            # ============ Phase C: dispatch metadata for own expert ============
            table = big.tile([128, KH, 5], f32)
            nc.sync.dma_start(table[:], rt_full[:].rearrange("(c p) f -> p c f", p=128))
            oh1 = sb.tile([128, KH], f32, tag="oh1")
            oh2 = sb.tile([128, KH], f32, tag="oh2")
            nc.vector.tensor_scalar(out=oh1[:], in0=table[:, :, 0], scalar1=eid_t[:],
                                    scalar2=None, op0=Alu.is_equal)
            nc.vector.tensor_scalar(out=oh2[:], in0=table[:, :, 1], scalar1=eid_t[:],
                                    scalar2=None, op0=Alu.is_equal)
            onehot = sb.tile([128, KH], f32, tag="onehot")
            nc.vector.tensor_add(onehot[:], oh1[:], oh2[:])
            w_e = sb.tile([128, KH], f32, tag="w_e")
            nc.vector.tensor_mul(oh1[:], oh1[:], table[:, :, 2])
            nc.vector.tensor_mul(oh2[:], oh2[:], table[:, :, 3])
            nc.vector.tensor_add(w_e[:], oh1[:], oh2[:])
            # exclusive prefix sum over token order (p-major): pos[p,c]
            incl = sb.tile([128, KH], f32, tag="incl")
            nc.vector.tensor_tensor_scan(incl[:], onehot[:], onehot[:], 0.0,
                                         op0=Alu.add, op1=Alu.bypass)
            rowsum = sb.tile([128, 1], f32, tag="rowsum")
            nc.vector.tensor_copy(rowsum[:], incl[:, KH - 1:KH])
            off_ps = psB.tile([128, 1], f32, tag="psmall", name="off_ps")
            nc.tensor.matmul(off_ps[:], tri[:], rowsum[:], start=True, stop=True)
            off_t = sb.tile([128, 1], f32, tag="off_t")
            nc.scalar.copy(off_t[:], off_ps[:])
            pos = sb.tile([128, KH], f32, tag="pos")
            nc.vector.tensor_scalar(out=pos[:], in0=incl[:], scalar1=off_t[:, :1],
                                    scalar2=None, op0=Alu.add)
            nc.vector.tensor_sub(pos[:], pos[:], onehot[:])
            # meta lhsT [128, c, 4]: (token id, weight, 1, r)
            meta = big.tile([128, KH, 4], f32r)
            ones_t = sb.tile([128, KH], f32, tag="ones_t")
            nc.vector.memset(ones_t[:], 1.0)
            nc.vector.tensor_copy(meta[:, :, 2], ones_t[:])
            nc.vector.tensor_copy(meta[:, :, 0], tval[:])
            nc.vector.tensor_copy(meta[:, :, 1], w_e[:])
            nc.vector.tensor_copy(meta[:, :, 3], table[:, :, 4])
            # meta_rows [4, CAP] = sum_c meta[:,c,:].T @ M_c
            mrow_ps = [psB.tile([4, CHW], f32, tag="psmall", name=f"mrow_ps{i}")
                       for i in range(SCH)]
            for c in range(KH):
                m_c = sb.tile([128, CAP], f32r, tag="m_c")
                nc.vector.tensor_scalar(out=m_c[:], in0=iob[:],
                                        scalar1=pos[:, c:c + 1],
                                        scalar2=onehot[:, c:c + 1],
                                        op0=Alu.is_equal, op1=Alu.mult)
                for i in range(SCH):
                    nc.tensor.matmul(mrow_ps[i][:], meta[:, c, :],
                                     m_c[:, i * CHW:(i + 1) * CHW],
                                     start=(c == 0), stop=(c == KH - 1))
            mrow = big.tile([4, CAP], f32)
            for i in range(SCH):
                nc.scalar.copy(mrow[:, i * CHW:(i + 1) * CHW], mrow_ps[i][:])
            # transpose to slot-major [128, st, 4]: cols 0=tok 1=w 2=mask 3=r
            smeta = big.tile([128, NST, 4], f32)
            nc.vector.memset(smeta[:], 0.0)
            for st in range(NST):
                w = ST_W[st]
                str_ps = psB.tile([128, 4], f32, tag="psmall", name="str_ps")
                nc.tensor.transpose(out=str_ps[:w, :],
                                    in_=mrow[:, st * 128:st * 128 + w],
                                    identity=ident[:4, :4])
                nc.vector.tensor_copy(smeta[:w, st, :], str_ps[:w, :])
            gidx = big.tile([128, NST], i32)       # gather index (token id)
            nc.vector.tensor_copy(gidx[:], smeta[:, :, 0])
            # scatter index: token id, or huge (skipped) for pad slots
            sidx_f = sb.tile([128, NST], f32, tag="sidx_f")
            nc.vector.tensor_scalar(out=sidx_f[:], in0=smeta[:, :, 2],
                                    scalar1=-1.0, scalar2=-3000000.0,
                                    op0=Alu.add, op1=Alu.mult)  # (mask-1)*-3e6
            nc.vector.tensor_add(sidx_f[:], sidx_f[:], smeta[:, :, 0])
            sidx = big.tile([128, NST], i32)
            nc.vector.tensor_copy(sidx[:], sidx_f[:])

            # ============ Phase D: gather + RMSNorm + transpose -> tnT ============
            tnT = big.tile([128, KH, CAP], f32r)
            for st in range(NST):
                g_t = sb.tile([128, H], f32, tag="scr8k", bufs=3, name="g_t")
                nc.gpsimd.indirect_dma_start(
                    out=g_t[:], out_offset=None, in_=x_d,
                    in_offset=bass.IndirectOffsetOnAxis(ap=gidx[:, st:st + 1], axis=0),
                    bounds_check=T - 1, oob_is_err=False)
                gn_t = sb.tile([128, H], f32, tag="scr8k", bufs=3, name="gn_t")
                nc.vector.scalar_tensor_tensor(gn_t[:], g_t[:],
                                               smeta[:, st, 3:4], nwb[:],
                                               op0=Alu.mult, op1=Alu.mult)
                w = ST_W[st]
                for k in range(KH):
                    ttr_ps = psA.tile([128, 128], f32, tag="pbig", name="ttr_ps")
                    nc.tensor.transpose(out=ttr_ps[:],
                                        in_=gn_t[:, k * 128:(k + 1) * 128],
                                        identity=ident[:])
                    nc.vector.tensor_copy(tnT[:, k, st * 128:st * 128 + w],
                                          ttr_ps[:, :w])

            # ============ Phase E: gate/up -> hT ============
            hT = big.tile([128, KI, CAP], f32r)
            for m in range(KI):
                wg_s = [None, None]
                wu_s = [None, None]
                for hf in range(2):
                    wg_s[hf] = wp.tile([128, KH // 2, 128], f32r, tag="wg_s",
                                       name=f"wg_s{hf}")
                    wu_s[hf] = wp.tile([128, KH // 2, 128], f32r, tag="wu_s",
                                       name=f"wu_s{hf}")
                    rows = slice(hf * (H // 2), (hf + 1) * (H // 2))
                    nc.sync.dma_start(
                        wg_s[hf][:], wg_d[rows, m * 128:(m + 1) * 128]
                        .rearrange("(k p) i -> p k i", p=128).bitcast(f32r))
                    nc.sync.dma_start(
                        wu_s[hf][:], wu_d[rows, m * 128:(m + 1) * 128]
                        .rearrange("(k p) i -> p k i", p=128).bitcast(f32r))
                for ch in range(SCH):
                    c0 = ch * CHW
                    g_ps = psA.tile([128, CHW], f32, tag="pbig", name="g_ps")
                    u_ps = psA.tile([128, CHW], f32, tag="pbig", name="u_ps")
                    for k in range(KH):
                        lg = wg_s[k // 8][:, k % 8, :]
                        lu = wu_s[k // 8][:, k % 8, :]
                        nc.tensor.matmul(g_ps[:], lg, tnT[:, k, c0:c0 + CHW],
                                         start=(k == 0), stop=(k == KH - 1))
                        nc.tensor.matmul(u_ps[:], lu, tnT[:, k, c0:c0 + CHW],
                                         start=(k == 0), stop=(k == KH - 1))
                    sg = sb.tile([128, CHW], f32, tag="sg")
                    nc.scalar.activation(sg[:], g_ps[:], Act.Silu)
                    nc.vector.tensor_mul(hT[:, m, c0:c0 + CHW], sg[:], u_ps[:])

            # ============ Phase F: down -> y chunks, scatter ============
            for n in range(NH):
                y_ps = [psA.tile([128, 512], f32, tag="pbig", name=f"y_ps{st}")
                        for st in range(NST)]
                for k in range(KI):
                    wd_t = wp.tile([128, 512], f32r, tag="wd_t", bufs=3)
                    nc.sync.dma_start(
                        wd_t[:], wd_d[k * 128:(k + 1) * 128,
                                      n * 512:(n + 1) * 512].bitcast(f32r))
                    for st in range(NST):
                        w = ST_W[st]
                        nc.tensor.matmul(y_ps[st][:w, :],
                                         hT[:, k, st * 128:st * 128 + w],
                                         wd_t[:], start=(k == 0), stop=(k == KI - 1))
                for st in range(NST):
                    w = ST_W[st]
                    y_ch = sb.tile([128, 512], f32, tag="y_ch")
                    nc.scalar.activation(y_ch[:w, :], y_ps[st][:w, :], Act.Copy,
                                         scale=smeta[:w, st, 1:2])
                    nc.gpsimd.indirect_dma_start(
                        out=contrib[:], out_offset=bass.IndirectOffsetOnAxis(
                            ap=sidx[:w, st:st + 1], axis=0),
                        in_=y_ch[:w, :], in_offset=None,
                        element_offset=n * 512,
                        bounds_check=T - 1, oob_is_err=False)

            # ============ Phase G: ReduceScatter + output ============
            nc.gpsimd.collective_compute("ReduceScatter", Alu.add,
                                         replica_groups=[list(range(NCORES))],
                                         ins=[contrib[:]], outs=[rs_out[:]])
            nc.sync.dma_start(out_d, rs_out[:])

    nc.compile()
    return nc


def _routing_counts(x2d, norm_w, router_w):
    t = x2d.astype(np.float64)
    r = 1.0 / np.sqrt((t * t).mean(-1, keepdims=True) + EPS)
    logits = (t * r * norm_w) @ router_w.astype(np.float64)
    order = np.argsort(-logits, axis=-1, kind="stable")
    top2 = order[:, :2]
    return np.bincount(top2.ravel(), minlength=E)


def kernel(x, norm_w, router_w, w_gate, w_up, w_down):
    from concourse.bass_utils import run_bass_kernel_spmd

    x = np.ascontiguousarray(np.asarray(x, dtype=np.float32))
    norm_w = np.ascontiguousarray(np.asarray(norm_w, dtype=np.float32))
    router_w = np.ascontiguousarray(np.asarray(router_w, dtype=np.float32))
    w_gate = np.asarray(w_gate, dtype=np.float32)
    w_up = np.asarray(w_up, dtype=np.float32)
    w_down = np.asarray(w_down, dtype=np.float32)

    x2d = x.reshape(T, H)
    counts = _routing_counts(x2d, norm_w, router_w)
    if counts.max() > CAP:
        raise RuntimeError(f"expert capacity {CAP} exceeded: counts={counts}")

    if "nc" not in _CACHE:
        _CACHE["nc"] = _build()
    nc = _CACHE["nc"]

    in_maps = []
    for c in range(NCORES):
        in_maps.append({
            "x": x2d,
            "x_slice": np.ascontiguousarray(x2d[c * TSL:(c + 1) * TSL]),
            "norm_w": norm_w,
            "router_w": router_w,
            "wg": np.ascontiguousarray(w_gate[c]),
            "wu": np.ascontiguousarray(w_up[c]),
            "wd": np.ascontiguousarray(w_down[c]),
            "eid": np.full((128, 1), float(c), dtype=np.float32),
        })
    res = run_bass_kernel_spmd(nc, in_maps, list(range(NCORES)))
    out = np.concatenate([res.results[c]["out_shard"] for c in range(NCORES)], axis=0)
    return out.reshape(B, S, H)



# revision 9
# speedup vs baseline: 1.2913x; 1.2913x over previous
"""MoE MLP block (RMSNorm + top-2 router + 8-expert GLU MLP) on 8 TRN2 cores.

Strategy: expert parallelism, one expert per core, AllToAll combine.
  - Every core computes the router for ALL tokens locally (x is replicated):
    RMSNorm stats from token-major x, logits from a host-pretransposed xT
    (so the PE does tiny N=8 matmuls instead of 256 128x128 transposes).
    This removes the routing AllGather and its entry-barrier skew.
  - Each core builds dispatch metadata for its own expert on-device
    (prefix-sum via DVE scan + strict-triangular matmul), indirect-DMA
    gathers its tokens' rows of x, applies RMSNorm, transposes to put H on
    partitions, and runs the expert GLU MLP in bf16 (weights quantized on
    host; f32 PSUM accumulation).
  - Combine: expert outputs are grouped by destination core (slot order =
    p-major within each 256-token destination slice, recomputable by the
    destination from the replicated routing table), scattered in bf16 into
    per-h-chunk AllToAll buffers, exchanged with 8 chunked AllToAlls that
    overlap the remaining down-proj compute, and each destination core
    gathers its two expert contributions per token and adds them in f32.
"""
import sys
sys.path.insert(0, '/opt/trn_rl_repo')
import numpy as np

# ---- problem constants (hardcoded per contract) ----
B, S, H, I, E = 2, 1024, 2048, 4096, 8
T = B * S                    # 2048 tokens
EPS = 1e-6
NCORES = 8
KH = H // 128                # 16 h-tiles
KI = I // 128                # 32 i-tiles
CAP = 560                    # max tokens per expert (seed-0 max count is 545;
                             # multiple of 16 keeps bf16/f32r slices 16B-aligned)
NST = (CAP + 127) // 128     # 5 slot tiles
ST_W = [min(128, CAP - st * 128) for st in range(NST)]   # 128,128,128,128,48
SCH = 2                      # gate/up slot chunks
CHW = CAP // SCH             # 280 per chunk
NH = 8                       # down-proj h chunks
HCW = H // NH                # 256 cols per chunk
TSL = T // NCORES            # 256 tokens per core's output slice
C2 = 96                      # per (expert, dest-core) capacity (seed-0 max 84)
R2 = E * C2                  # 768 rows in each AllToAll buffer

_CACHE = {}


def _build():
    from concourse import bass, mybir
    import concourse.bacc as bacc
    import concourse.tile as tile
    from concourse.masks import make_identity

    dt = mybir.dt
    f32, f32r, i32, u32 = dt.float32, dt.float32r, dt.int32, dt.uint32
    bf16 = dt.bfloat16
    Alu = mybir.AluOpType
    Act = mybir.ActivationFunctionType

    nc = bacc.Bacc("TRN2", target_bir_lowering=False, debug=False,
                   num_devices=NCORES)

    x_d = nc.dram_tensor("x", [T, H], f32, kind="ExternalInput").ap()
    # host-pretransposed x: [128, KH, T] with xt[p, k, t] = x[t, k*128+p]
    xt_d = nc.dram_tensor("xt", [128, KH, T], f32, kind="ExternalInput").ap()
    nw_d = nc.dram_tensor("norm_w", [H], f32, kind="ExternalInput").ap()
    rw_d = nc.dram_tensor("router_w", [H, E], f32, kind="ExternalInput").ap()
    # gate/up pretiled to [128, KI, KH, 128] bf16: per-m DMA = 4KB line/part
    wg_d = nc.dram_tensor("wg", [128, KI, KH, 128], bf16, kind="ExternalInput").ap()
    wu_d = nc.dram_tensor("wu", [128, KI, KH, 128], bf16, kind="ExternalInput").ap()
    # down pretiled to [128, NH, KI, HCW] bf16: per-(n,khalf) DMA = 8KB line
    wd_d = nc.dram_tensor("wd", [128, NH, KI, HCW], bf16, kind="ExternalInput").ap()
    eid_d = nc.dram_tensor("eid", [128, 1], f32, kind="ExternalInput").ap()
    out_d = nc.dram_tensor("out_shard", [TSL, H], f32, kind="ExternalOutput").ap()

    with tile.TileContext(nc) as tc:
        with tc.tile_pool(name="cst", bufs=1) as cst, \
             tc.tile_pool(name="sb", bufs=2) as sb, \
             tc.tile_pool(name="big", bufs=1) as big, \
             tc.tile_pool(name="wp", bufs=2) as wp, \
             tc.tile_pool(name="psA", bufs=6, space="PSUM") as psA, \
             tc.tile_pool(name="psB", bufs=2, space="PSUM") as psB, \
             tc.tile_pool(name="dram", bufs=1, space="DRAM") as dram:

            # ============ DRAM scratch: chunked AllToAll buffers (bf16) ======
            a2a_src = [dram.tile([R2, HCW], bf16, name=f"a2a_src{n}")
                       for n in range(NH)]
            a2a_dst = [dram.tile([R2, HCW], bf16, name=f"a2a_dst{n}")
                       for n in range(NH)]

            # ============ constants ============
            ident = cst.tile([128, 128], f32)
            make_identity(nc, ident[:])
            tri = cst.tile([128, 128], f32)        # tri[p',p]=1 iff p'<p
            nc.gpsimd.memset(tri[:], 1.0)
            nc.gpsimd.affine_select(out=tri[:], in_=tri[:], compare_op=Alu.is_gt,
                                    fill=0.0, base=0, pattern=[[1, 128]],
                                    channel_multiplier=-1)
            eid_t = cst.tile([128, 1], f32)
            nc.sync.dma_start(eid_t[:], eid_d)
            nwb = cst.tile([128, H], f32)          # norm_w broadcast to 128 parts
            nc.sync.dma_start(nwb[:], nw_d.unsqueeze(0).to_broadcast([128, H]))
            iob = cst.tile([128, CAP], f32)        # each row = 0..CAP-1
            nc.gpsimd.iota(iob[:].bitcast(i32), pattern=[[1, CAP]], base=0,
                           channel_multiplier=0)
            nc.vector.tensor_copy(iob[:], iob[:].bitcast(i32))
            iob8 = cst.tile([128, E], f32)         # each row = 0..7
            nc.gpsimd.iota(iob8[:].bitcast(i32), pattern=[[1, E]], base=0,
                           channel_multiplier=0)
            nc.vector.tensor_copy(iob8[:], iob8[:].bitcast(i32))
            colidx = cst.tile([128, KH], f32)      # each row = 0..15
            nc.gpsimd.iota(colidx[:].bitcast(i32), pattern=[[1, KH]], base=0,
                           channel_multiplier=0)
            nc.vector.tensor_copy(colidx[:], colidx[:].bitcast(i32))
            tval = cst.tile([128, KH], f32)        # token id at (p, c): c*128+p
            nc.gpsimd.iota(tval[:].bitcast(i32), pattern=[[128, KH]], base=0,
                           channel_multiplier=1)
            nc.vector.tensor_copy(tval[:], tval[:].bitcast(i32))
            # router weight folded with norm_w
            rw_t = sb.tile([128, KH, E], f32, tag="rw_t")
            nc.sync.dma_start(rw_t[:], rw_d.rearrange("(k p) e -> p k e", p=128))
            nw_t = sb.tile([128, KH], f32, tag="nw_t")
            nc.sync.dma_start(nw_t[:], nw_d.rearrange("(k p) -> p k", p=128))
            wp_t = cst.tile([128, KH, E], f32)
            for k in range(KH):
                nc.vector.tensor_scalar(out=wp_t[:, k, :], in0=rw_t[:, k, :],
                                        scalar1=nw_t[:, k:k + 1], scalar2=None,
                                        op0=Alu.mult)

            # ============ Phase A: router for ALL tokens (replicated) ========
            # table columns: 0=e1 1=e2 2=w1 3=w2 4=r
            table = big.tile([128, KH, 5], f32)
            # A1: RMSNorm scale r per token, from token-major x
            for j in range(KH):
                xsj = sb.tile([128, H], f32, tag="scr8k", bufs=3, name="xsj")
                nc.sync.dma_start(xsj[:], x_d[j * 128:(j + 1) * 128, :])
                sq_scr = sb.tile([128, H], f32, tag="scr8k", bufs=3, name="sq_scr")
                ssq = sb.tile([128, 1], f32, tag="ssq")
                nc.scalar.activation(sq_scr[:], xsj[:], Act.Square, accum_out=ssq[:])
                var = sb.tile([128, 1], f32, tag="var")
                nc.vector.tensor_scalar(out=var[:], in0=ssq[:], scalar1=1.0 / H,
                                        scalar2=float(EPS), op0=Alu.mult, op1=Alu.add)
                sd = sb.tile([128, 1], f32, tag="sd")
                nc.scalar.sqrt(sd[:], var[:])
                r_col = sb.tile([128, 1], f32, tag="r_col")
                nc.vector.reciprocal(r_col[:], sd[:])
                nc.vector.tensor_copy(table[:, j, 4:5], r_col[:])
            # A2: logitsT [8, T] via pretransposed xT tiles; 4 PSUM chunk
            # tiles accumulate over k (bank-granular groups, like gate/up)
            lgq = [psA.tile([8, 512], f32, tag="pbig", name=f"lgq{q}")
                   for q in range(4)]
            for k in range(KH):
                xtk = sb.tile([128, T], f32, tag="scr8k", bufs=3, name="xtk")
                nc.sync.dma_start(xtk[:], xt_d[:, k, :])
                for q in range(4):
                    nc.tensor.matmul(lgq[q][:], wp_t[:, k, :],
                                     xtk[:, q * 512:(q + 1) * 512],
                                     start=(k == 0), stop=(k == KH - 1))
            lgT = sb.tile([8, T], f32, tag="lgT")
            for q in range(4):
                nc.scalar.copy(lgT[:, q * 512:(q + 1) * 512], lgq[q][:])
            # A3: per j-tile: transpose [8,128] back to token-major, scale by
            # r, top-2, weights
            for j in range(KH):
                ltr_ps = psB.tile([128, E], f32, tag="psmall", name="ltr_ps")
                nc.tensor.transpose(out=ltr_ps[:],
                                    in_=lgT[:, j * 128:(j + 1) * 128],
                                    identity=ident[:8, :8])
                s_t = sb.tile([128, E], f32, tag="s_t")
                nc.scalar.activation(s_t[:], ltr_ps[:], Act.Copy,
                                     scale=table[:, j, 4:5])
                mx = sb.tile([128, 8], f32, tag="mx")
                mi = sb.tile([128, 8], u32, tag="mi")
                nc.vector.max_with_indices(mx[:], mi[:], s_t[:])
                dlt = sb.tile([128, 1], f32, tag="dlt")
                nc.vector.tensor_sub(dlt[:], mx[:, 1:2], mx[:, 0:1])
                ew = sb.tile([128, 1], f32, tag="ew")
                nc.scalar.activation(ew[:], dlt[:], Act.Exp)
                den = sb.tile([128, 1], f32, tag="den")
                nc.vector.tensor_scalar_add(den[:], ew[:], 1.0)
                w1 = sb.tile([128, 1], f32, tag="w1")
                nc.vector.reciprocal(w1[:], den[:])
                nc.vector.tensor_copy(table[:, j, 0:2], mi[:, 0:2])
                nc.vector.tensor_copy(table[:, j, 2:3], w1[:])
                nc.vector.tensor_mul(table[:, j, 3:4], ew[:], w1[:])

            # ============ Phase B: combine indices for MY output slice =======
            # Extract my two table columns (2c, 2c+1) by value using eid.
            rt_own = big.tile([128, 2, 5], f32)
            junk16 = sb.tile([128, KH], f32, tag="junk16")
            junk16b = sb.tile([128, KH], f32, tag="junk16b")
            for j in range(2):
                eid2 = sb.tile([128, 1], f32, tag="eid2")
                nc.vector.tensor_scalar(out=eid2[:], in0=eid_t[:], scalar1=2.0,
                                        scalar2=float(j), op0=Alu.mult, op1=Alu.add)
                mcol = sb.tile([128, KH], f32, tag="mcol")
                nc.vector.tensor_scalar(out=mcol[:], in0=colidx[:],
                                        scalar1=eid2[:, :1], scalar2=None,
                                        op0=Alu.is_equal)
                for f in range(5):
                    nc.vector.tensor_mul(junk16[:], table[:, :, f], mcol[:])
                    nc.scalar.activation(junk16b[:], junk16[:], Act.Copy,
                                         accum_out=rt_own[:, j, f:f + 1])
            # per-expert counts over my token pair-slice, p-major prefix
            junk8 = sb.tile([128, E], f32, tag="junk8")
            junk8b = sb.tile([128, E], f32, tag="junk8b")
            m_sel = [[None, None], [None, None]]   # [j][k] masks
            cnt_j = [None, None]
            for j in range(2):
                for kk in range(2):
                    m = sb.tile([128, E], f32, tag=f"msel{j}{kk}")
                    nc.vector.tensor_scalar(out=m[:], in0=iob8[:],
                                            scalar1=rt_own[:, j, kk:kk + 1],
                                            scalar2=None, op0=Alu.is_equal)
                    m_sel[j][kk] = m
                c = sb.tile([128, E], f32, tag=f"cnt{j}")
                nc.vector.tensor_add(c[:], m_sel[j][0][:], m_sel[j][1][:])
                cnt_j[j] = c
            paircnt = sb.tile([128, E], f32, tag="paircnt")
            nc.vector.tensor_add(paircnt[:], cnt_j[0][:], cnt_j[1][:])
            offd_ps = psB.tile([128, E], f32, tag="psmall", name="offd_ps")
            nc.tensor.matmul(offd_ps[:], tri[:], paircnt[:], start=True, stop=True)
            offd0 = sb.tile([128, E], f32, tag="offd0")
            nc.scalar.copy(offd0[:], offd_ps[:])
            offd1 = sb.tile([128, E], f32, tag="offd1")
            nc.vector.tensor_add(offd1[:], offd0[:], cnt_j[0][:])
            offd = [offd0, offd1]
            # ridx columns: [j0e1, j0e2, j1e1, j1e2]
            ridx = big.tile([128, 4], i32)
            for j in range(2):
                for kk in range(2):
                    svec = sb.tile([128, 1], f32, tag="svec")
                    nc.vector.tensor_mul(junk8[:], m_sel[j][kk][:], offd[j][:])
                    nc.scalar.activation(junk8b[:], junk8[:], Act.Copy,
                                         accum_out=svec[:])
                    rf = sb.tile([128, 1], f32, tag="rf")
                    nc.vector.tensor_scalar(out=rf[:], in0=rt_own[:, j, kk:kk + 1],
                                            scalar1=float(C2), scalar2=None,
                                            op0=Alu.mult)
                    nc.vector.tensor_add(rf[:], rf[:], svec[:])
                    nc.vector.tensor_copy(ridx[:, 2 * j + kk:2 * j + kk + 1], rf[:])

            # ============ Phase C: dispatch metadata for own expert ==========
            oh1 = sb.tile([128, KH], f32, tag="oh1")
            oh2 = sb.tile([128, KH], f32, tag="oh2")
            nc.vector.tensor_scalar(out=oh1[:], in0=table[:, :, 0], scalar1=eid_t[:],
                                    scalar2=None, op0=Alu.is_equal)
            nc.vector.tensor_scalar(out=oh2[:], in0=table[:, :, 1], scalar1=eid_t[:],
                                    scalar2=None, op0=Alu.is_equal)
            onehot = sb.tile([128, KH], f32, tag="onehot")
            nc.vector.tensor_add(onehot[:], oh1[:], oh2[:])
            w_e = sb.tile([128, KH], f32, tag="w_e")
            nc.vector.tensor_mul(oh1[:], oh1[:], table[:, :, 2])
            nc.vector.tensor_mul(oh2[:], oh2[:], table[:, :, 3])
            nc.vector.tensor_add(w_e[:], oh1[:], oh2[:])
            # exclusive prefix sum over token order (p-major): pos[p,c]
            incl = sb.tile([128, KH], f32, tag="incl")
            nc.vector.tensor_tensor_scan(incl[:], onehot[:], onehot[:], 0.0,
                                         op0=Alu.add, op1=Alu.bypass)
            rowsum = sb.tile([128, 1], f32, tag="rowsum")
            nc.vector.tensor_copy(rowsum[:], incl[:, KH - 1:KH])
            off_ps = psB.tile([128, 1], f32, tag="psmall", name="off_ps")
            nc.tensor.matmul(off_ps[:], tri[:], rowsum[:], start=True, stop=True)
            off_t = sb.tile([128, 1], f32, tag="off_t")
            nc.scalar.copy(off_t[:], off_ps[:])
            pos = sb.tile([128, KH], f32, tag="pos")
            nc.vector.tensor_scalar(out=pos[:], in0=incl[:], scalar1=off_t[:, :1],
                                    scalar2=None, op0=Alu.add)
            nc.vector.tensor_sub(pos[:], pos[:], onehot[:])
            # a2a destination row for each of my expert's tokens:
            #   row = dest*C2 + (p-major rank within the dest's column pair)
            ohr = onehot[:].rearrange("p (d j) -> p d j", j=2)
            pairsum = sb.tile([128, E], f32, tag="pairsum")
            nc.vector.tensor_add(pairsum[:], ohr[:, :, 0], ohr[:, :, 1])
            offp_ps = psB.tile([128, E], f32, tag="psmall", name="offp_ps")
            nc.tensor.matmul(offp_ps[:], tri[:], pairsum[:], start=True, stop=True)
            base8 = sb.tile([128, E], f32, tag="base8")
            nc.scalar.copy(base8[:], offp_ps[:])
            dC2 = sb.tile([128, E], f32, tag="dC2")
            nc.vector.tensor_scalar(out=dC2[:], in0=iob8[:], scalar1=float(C2),
                                    scalar2=None, op0=Alu.mult)
            nc.vector.tensor_add(base8[:], base8[:], dC2[:])
            a2a16 = sb.tile([128, KH], f32, tag="a2a16")
            ar = a2a16[:].rearrange("p (d j) -> p d j", j=2)
            nc.vector.tensor_copy(ar[:, :, 0], base8[:])
            nc.vector.tensor_add(ar[:, :, 1], base8[:], ohr[:, :, 0])
            # meta lhsT [128, c, 8]: (token id, weight, 1, r, a2a row, 0,0,0)
            # (8 cols, not 5, so f32r matmul operand offsets stay 16B-aligned)
            meta = big.tile([128, KH, 8], f32r)
            nc.vector.memset(meta[:].bitcast(f32), 0.0)
            ones_t = sb.tile([128, KH], f32, tag="ones_t")
            nc.vector.memset(ones_t[:], 1.0)
            nc.vector.tensor_copy(meta[:, :, 2], ones_t[:])
            nc.vector.tensor_copy(meta[:, :, 0], tval[:])
            nc.vector.tensor_copy(meta[:, :, 1], w_e[:])
            nc.vector.tensor_copy(meta[:, :, 3], table[:, :, 4])
            nc.vector.tensor_copy(meta[:, :, 4], a2a16[:])
            # meta_rows [8, CAP] = sum_c meta[:,c,:].T @ M_c
            mrow_ps = [psB.tile([8, CHW], f32, tag="psmall", name=f"mrow_ps{i}")
                       for i in range(SCH)]
            for c in range(KH):
                m_c = sb.tile([128, CAP], f32r, tag="m_c")
                nc.vector.tensor_scalar(out=m_c[:], in0=iob[:],
                                        scalar1=pos[:, c:c + 1],
                                        scalar2=onehot[:, c:c + 1],
                                        op0=Alu.is_equal, op1=Alu.mult)
                for i in range(SCH):
                    nc.tensor.matmul(mrow_ps[i][:], meta[:, c, :],
                                     m_c[:, i * CHW:(i + 1) * CHW],
                                     start=(c == 0), stop=(c == KH - 1))
            mrow = big.tile([8, CAP], f32)
            for i in range(SCH):
                nc.scalar.copy(mrow[:, i * CHW:(i + 1) * CHW], mrow_ps[i][:])
            # transpose to slot-major [128, st, 8]: 0=tok 1=w 2=mask 3=r 4=a2a
            smeta = big.tile([128, NST, 8], f32)
            nc.vector.memset(smeta[:], 0.0)
            for st in range(NST):
                w = ST_W[st]
                str_ps = psB.tile([128, 8], f32, tag="psmall", name="str_ps")
                nc.tensor.transpose(out=str_ps[:w, :],
                                    in_=mrow[:, st * 128:st * 128 + w],
                                    identity=ident[:8, :8])
                nc.vector.tensor_copy(smeta[:w, st, :], str_ps[:w, :])
            gidx = big.tile([128, NST], i32)       # gather index (token id)
            nc.vector.tensor_copy(gidx[:], smeta[:, :, 0])
            # scatter index: a2a row, or huge (skipped) for pad slots
            sidx_f = sb.tile([128, NST], f32, tag="sidx_f")
            nc.vector.tensor_scalar(out=sidx_f[:], in0=smeta[:, :, 2],
                                    scalar1=-1.0, scalar2=-3000000.0,
                                    op0=Alu.add, op1=Alu.mult)  # (mask-1)*-3e6
            nc.vector.tensor_add(sidx_f[:], sidx_f[:], smeta[:, :, 4])
            sidx = big.tile([128, NST], i32)
            nc.vector.tensor_copy(sidx[:], sidx_f[:])

            # ============ Phase D: gather + RMSNorm + transpose -> tnT =======
            tnT = big.tile([128, KH, CAP], bf16)
            for st in range(NST):
                g_t = sb.tile([128, H], f32, tag="scr8k", bufs=3, name="g_t")
                nc.gpsimd.indirect_dma_start(
                    out=g_t[:], out_offset=None, in_=x_d,
                    in_offset=bass.IndirectOffsetOnAxis(ap=gidx[:, st:st + 1], axis=0),
                    bounds_check=T - 1, oob_is_err=False)
                gn_t = sb.tile([128, H], f32, tag="scr8k", bufs=3, name="gn_t")
                nc.vector.scalar_tensor_tensor(gn_t[:], g_t[:],
                                               smeta[:, st, 3:4], nwb[:],
                                               op0=Alu.mult, op1=Alu.mult)
                w = ST_W[st]
                for k in range(KH):
                    ttr_ps = psA.tile([128, 128], f32, tag="pbig", name="ttr_ps")
                    nc.tensor.transpose(out=ttr_ps[:],
                                        in_=gn_t[:, k * 128:(k + 1) * 128],
                                        identity=ident[:])
                    nc.vector.tensor_copy(tnT[:, k, st * 128:st * 128 + w],
                                          ttr_ps[:, :w])

            # ============ Phase E: gate/up -> hT (bf16) ============
            hT = big.tile([128, KI, CAP], bf16)
            for m in range(KI):
                wgm = wp.tile([128, KH, 128], bf16, tag="wgm")
                wum = wp.tile([128, KH, 128], bf16, tag="wum")
                nc.sync.dma_start(wgm[:], wg_d[:, m])
                nc.sync.dma_start(wum[:], wu_d[:, m])
                g_ps = [psA.tile([128, CHW], f32, tag="pbig", name=f"g_ps{ch}")
                        for ch in range(SCH)]
                u_ps = [psA.tile([128, CHW], f32, tag="pbig", name=f"u_ps{ch}")
                        for ch in range(SCH)]
                for k in range(KH):
                    for ch in range(SCH):
                        nc.tensor.matmul(g_ps[ch][:], wgm[:, k, :],
                                         tnT[:, k, ch * CHW:(ch + 1) * CHW],
                                         start=(k == 0), stop=(k == KH - 1))
                    for ch in range(SCH):
                        nc.tensor.matmul(u_ps[ch][:], wum[:, k, :],
                                         tnT[:, k, ch * CHW:(ch + 1) * CHW],
                                         start=(k == 0), stop=(k == KH - 1))
                for ch in range(SCH):
                    sg = sb.tile([128, CHW], f32, tag="sg")
                    nc.scalar.activation(sg[:], g_ps[ch][:], Act.Silu)
                    nc.vector.tensor_mul(hT[:, m, ch * CHW:(ch + 1) * CHW],
                                         sg[:], u_ps[ch][:])

            # ============ Phase F: down -> y chunks, scatter + chunked A2A ===
            def combine(n):
                for j in range(2):
                    g1 = sb.tile([128, HCW], bf16, tag="cmbg", bufs=4, name="g1")
                    g2 = sb.tile([128, HCW], bf16, tag="cmbg", bufs=4, name="g2")
                    nc.gpsimd.indirect_dma_start(
                        out=g1[:], out_offset=None, in_=a2a_dst[n][:],
                        in_offset=bass.IndirectOffsetOnAxis(
                            ap=ridx[:, 2 * j:2 * j + 1], axis=0),
                        bounds_check=R2 - 1, oob_is_err=False)
                    nc.gpsimd.indirect_dma_start(
                        out=g2[:], out_offset=None, in_=a2a_dst[n][:],
                        in_offset=bass.IndirectOffsetOnAxis(
                            ap=ridx[:, 2 * j + 1:2 * j + 2], axis=0),
                        bounds_check=R2 - 1, oob_is_err=False)
                    o_t = sb.tile([128, HCW], f32, tag="cmbo", bufs=4, name="o_t")
                    nc.vector.tensor_add(o_t[:], g1[:], g2[:])
                    nc.sync.dma_start(
                        out_d[j * 128:(j + 1) * 128, n * HCW:(n + 1) * HCW], o_t[:])

            for n in range(NH):
                y_ps = [psA.tile([128, HCW], f32, tag="pbig", name=f"y_ps{st}")
                        for st in range(NST)]
                for kh in range(2):
                    wd_t = wp.tile([128, KI // 2, HCW], bf16, tag="wd_t", bufs=3)
                    nc.sync.dma_start(wd_t[:], wd_d[:, n, kh * 16:(kh + 1) * 16, :])
                    for k2 in range(KI // 2):
                        k = kh * 16 + k2
                        for st in range(NST):
                            w = ST_W[st]
                            nc.tensor.matmul(y_ps[st][:w, :],
                                             hT[:, k, st * 128:st * 128 + w],
                                             wd_t[:, k2, :],
                                             start=(k == 0), stop=(k == KI - 1))
                for st in range(NST):
                    w = ST_W[st]
                    y_ch = sb.tile([128, HCW], bf16, tag="y_ch")
                    nc.scalar.activation(y_ch[:w, :], y_ps[st][:w, :], Act.Copy,
                                         scale=smeta[:w, st, 1:2])
                    nc.gpsimd.indirect_dma_start(
                        out=a2a_src[n][:], out_offset=bass.IndirectOffsetOnAxis(
                            ap=sidx[:w, st:st + 1], axis=0),
                        in_=y_ch[:w, :], in_offset=None,
                        bounds_check=R2 - 1, oob_is_err=False)
                nc.gpsimd.collective_compute(
                    "AllToAll", Alu.bypass,
                    replica_groups=[list(range(NCORES))],
                    ins=[a2a_src[n][:]], outs=[a2a_dst[n][:]])
                if n >= 2:
                    combine(n - 2)
            combine(NH - 2)
            combine(NH - 1)

    nc.compile()
    return nc


def _routing_counts(x2d, norm_w, router_w):
    t = x2d.astype(np.float64)
    r = 1.0 / np.sqrt((t * t).mean(-1, keepdims=True) + EPS)
    logits = (t * r * norm_w) @ router_w.astype(np.float64)
    order = np.argsort(-logits, axis=-1, kind="stable")
    top2 = order[:, :2]
    counts = np.bincount(top2.ravel(), minlength=E)
    dest = np.arange(x2d.shape[0]) // TSL
    pair = np.zeros((E, NCORES), dtype=np.int64)
    np.add.at(pair, (top2[:, 0], dest), 1)
    np.add.at(pair, (top2[:, 1], dest), 1)
    return counts, pair


def _bf16(a):
    import ml_dtypes
    return a.astype(ml_dtypes.bfloat16)


def _pretile_gate_up(w):
    # [H, I] -> [128, KI, KH, 128] bf16
    return _bf16(np.ascontiguousarray(
        w.reshape(KH, 128, KI, 128).transpose(1, 2, 0, 3)))


def _pretile_down(w):
    # [I, H] -> [128, NH, KI, HCW] bf16
    return _bf16(np.ascontiguousarray(
        w.reshape(KI, 128, NH, HCW).transpose(1, 2, 0, 3)))


def _pretile_xt(x2d):
    # [T, H] -> [128, KH, T] f32 with xt[p, k, t] = x[t, k*128+p]
    return np.ascontiguousarray(x2d.T.reshape(KH, 128, T).transpose(1, 0, 2))


def _make_in_maps(inputs):
    x = np.ascontiguousarray(np.asarray(inputs["x"], dtype=np.float32))
    norm_w = np.ascontiguousarray(np.asarray(inputs["norm_w"], dtype=np.float32))
    router_w = np.ascontiguousarray(np.asarray(inputs["router_w"], dtype=np.float32))
    w_gate = np.asarray(inputs["w_gate"], dtype=np.float32)
    w_up = np.asarray(inputs["w_up"], dtype=np.float32)
    w_down = np.asarray(inputs["w_down"], dtype=np.float32)
    x2d = x.reshape(T, H)
    xt = _pretile_xt(x2d)
    in_maps = []
    for c in range(NCORES):
        in_maps.append({
            "x": x2d,
            "xt": xt,
            "norm_w": norm_w,
            "router_w": router_w,
            "wg": _pretile_gate_up(w_gate[c]),
            "wu": _pretile_gate_up(w_up[c]),
            "wd": _pretile_down(w_down[c]),
            "eid": np.full((128, 1), float(c), dtype=np.float32),
        })
    return in_maps


def kernel(x, norm_w, router_w, w_gate, w_up, w_down):
    from concourse.bass_utils import run_bass_kernel_spmd

    inputs = {"x": x, "norm_w": norm_w, "router_w": router_w,
              "w_gate": w_gate, "w_up": w_up, "w_down": w_down}
    x2d = np.asarray(x, dtype=np.float32).reshape(T, H)
    counts, pair = _routing_counts(
        x2d, np.asarray(norm_w, np.float32), np.asarray(router_w, np.float32))
    if counts.max() > CAP:
        raise RuntimeError(f"expert capacity {CAP} exceeded: counts={counts}")
    if pair.max() > C2:
        raise RuntimeError(f"a2a capacity {C2} exceeded: max={pair.max()}")

    if "nc" not in _CACHE:
        _CACHE["nc"] = _build()
    nc = _CACHE["nc"]

    in_maps = _make_in_maps(inputs)
    res = run_bass_kernel_spmd(nc, in_maps, list(range(NCORES)))
    out = np.concatenate([res.results[c]["out_shard"] for c in range(NCORES)], axis=0)
    return out.reshape(B, S, H)
